# revision 1
# baseline (speedup 1.0000x reference)
"""LocalFeatureAggregation Trainium2 kernel (8 NeuronCores, data-parallel over nodes).

Architecture ("logit-table + segment-matmul"):

  Math identity: with BN folded (g = a*x + c), the attention logits
  L[n,k,:] = g[n,k,:] @ Ws depend only on the SOURCE node j = idx[n,k]:
      L[j] = x[j] @ (a*Ws) + c@Ws
      feat[n] = a * (sum_k P[j]) / (sum_k E[j]) + c,  E = exp(L), P = E*x
      out[n]  = (sum_k P / sum_k E) @ (a*Wm) + (c@Wm + bm)
  so all exp/matmul work is PER NODE (50k) instead of PER EDGE (800k).

  Phase 1 (replicated on each core, channel-major):
      x = relu(f@W1 + b1)   [DVE evac, bias fused]
      L = x @ Ws' (+cs bias fused into exp)  -> E = exp(L)  [Act]
      P = E * x  [Pool]
      4 PE transposes assemble node-major rows [E(256) | P(256)], evac to
      SBUF as fp8 -> DRAM table (fp8 e4m3: validated |L|<3, P<50, no overflow).

  Phase 2 (per core, 49 sups of 128 destination nodes):
      dma_gather (transpose=False) fetches each edge's 512B table row onto
      partition slot%128. int16 index limit + the HW ~1024-idx/call ring cap
      are handled by THREE subtables (rows [0,16768), [16768,33536),
      [33536,50176)) with per-sup compact slot lists padded to 128 multiples
      (pad idx=0, killed by zero columns in S).
      Aggregation sum_k via one-hot segment matmuls: lhsT = S[slot, node]
      (host-built 0/1), rhs = gathered rows -> PSUM [128 nodes, 512 f32].
      Then t = P_sum * recip(E_sum) [DVE], PE-transpose t, final matmul with
      wmp = a*Wm, bias bmp = c@Wm + bm added on DVE, DMA out.
"""

import os

import numpy as np
import ml_dtypes

import concourse.bass as bass
import concourse.bacc as bacc
import concourse.tile as tile
from concourse import mybir
from concourse.bass_utils import run_bass_kernel_spmd

BN_EPS = 1e-5
P = 128
N_NODES = 50000
K_NBR = 16
C_IN = 128
C2 = 256
C_OUT = 128
N_CORES = 8
NODES_PC_RAW = N_NODES // N_CORES            # 6250
NT = 50176                                   # padded table rows (392*128)
TBOUND = (0, 16768, 33536, NT)               # 3 subtables, each < 32768 rows
NSUB = 3
SUP = 128                                    # dest nodes per sup
EPS = 2048                                   # edges per sup
FBLK = 512                                   # phase-1 feature strip columns (non-shard)
GCAP = 1024                                  # HW cap on idxs per dma_gather

BF16 = mybir.dt.bfloat16
F32 = mybir.dt.float32
FP8 = mybir.dt.float8e4

TAB_FP8 = True                               # table dtype switch
PHASES = os.environ.get("KPHASES", "12")
TAB_KIND = os.environ.get("KTABKIND", "Internal")
KREPEAT = int(os.environ.get("KREPEAT", "1"))  # loop body N times (timing only)
DROW = os.environ.get("KDROW", "1") == "1" and TAB_FP8  # DoubleRow fp8 agg
KSHARD = os.environ.get("KSHARD", "0") == "1"  # shard phase1 + AllGather
NTS = NT // N_CORES                            # 6272 table rows per core shard
KDENSE = os.environ.get("KDENSE", "0") == "1"  # timing: dense DMA replaces gathers
KNOTAIL = os.environ.get("KNOTAIL", "0") == "1"  # timing: skip divide/final tail

_PROG_CACHE = {}


def build_program(nsup, sizes):
    """sizes: [NSUB][nsup] padded slot counts (multiples of 128, <= GCAP)."""
    tdt = FP8 if TAB_FP8 else BF16
    nodes_pc = nsup * SUP
    nc = bacc.Bacc("TRN2", num_devices=N_CORES, num_swdge_queues=3)

    p1n = NTS if KSHARD else NT              # phase-1 nodes computed per core
    fcm_d = nc.dram_tensor("fcm", [P, p1n], BF16, kind="ExternalInput")
    w1_d = nc.dram_tensor("w1", [C_IN, C2], BF16, kind="ExternalInput")
    ws_d = nc.dram_tensor("wsp", [C2, C2], BF16, kind="ExternalInput")
    wm_d = nc.dram_tensor("wmp", [C2, C_OUT], BF16, kind="ExternalInput")
    b1_d = nc.dram_tensor("b1c", [P, 2], F32, kind="ExternalInput")
    cs_d = nc.dram_tensor("csc", [P, 2], F32, kind="ExternalInput")
    bm_d = nc.dram_tensor("bmr", [1, C_OUT], F32, kind="ExternalInput")
    idn_d = nc.dram_tensor("idn", [P, P], BF16, kind="ExternalInput")

    tot_chunks = sum(sizes[t][s] for t in range(NSUB) for s in range(nsup)) // P
    idx_tot = tot_chunks * P
    idx_d = nc.dram_tensor("idxs", [P, idx_tot // 16], mybir.dt.int16,
                           kind="ExternalInput")
    s_d = nc.dram_tensor("smat", [P, tot_chunks * P], tdt, kind="ExternalInput")

    if TAB_KIND == "Internal":
        tab_d = nc.dram_tensor("tab", [NT, 2 * C2], tdt, kind="Internal")
    elif TAB_KIND == "ExternalOutput":
        tab_d = nc.dram_tensor("tab", [NT, 2 * C2], tdt, kind="ExternalOutput")
    else:
        tab_d = nc.dram_tensor("tab", [NT, 2 * C2], tdt, kind="ExternalInput")
    tabs_d = (nc.dram_tensor("tabs", [NTS, 2 * C2], tdt, kind="Internal")
              if KSHARD else tab_d)
    out_d = nc.dram_tensor("out", [nodes_pc, C_OUT], F32, kind="ExternalOutput")

    with tile.TileContext(nc) as tc:
        with (
            tc.tile_pool(name="consts", bufs=1) as consts,
            tc.tile_pool(name="fpool", bufs=2) as fpool,
            tc.tile_pool(name="p1w", bufs=3) as p1w,
            tc.tile_pool(name="gpool", bufs=4) as gpool,
            tc.tile_pool(name="spool", bufs=2) as spool,
            tc.tile_pool(name="p2w", bufs=3) as p2w,
            tc.tile_pool(name="tpool", bufs=1) as tpool,
            tc.tile_pool(name="opool", bufs=3) as opool,
        ):
            # ---- constants -----------------------------------------------
            w1s = consts.tile([P, C2], BF16)
            nc.sync.dma_start(out=w1s, in_=w1_d[:, :])
            wss = consts.tile([P, 2, C2], BF16)       # [k-chunk][ch', free]
            for k in range(2):
                nc.sync.dma_start(out=wss[:, k, :], in_=ws_d[k * P:(k + 1) * P, :])
            wms = consts.tile([P, 2, C_OUT], BF16)
            for k in range(2):
                nc.sync.dma_start(out=wms[:, k, :], in_=wm_d[k * P:(k + 1) * P, :])
            b1c = consts.tile([P, 2], F32)
            nc.sync.dma_start(out=b1c, in_=b1_d[:, :])
            csc = consts.tile([P, 2], F32)
            nc.sync.dma_start(out=csc, in_=cs_d[:, :])
            bmb = consts.tile([P, C_OUT], F32)
            nc.sync.dma_start(
                out=bmb, in_=bass.AP(tensor=bm_d, offset=0, ap=[[0, P], [1, C_OUT]])
            )
            idn = consts.tile([P, P], BF16)
            nc.sync.dma_start(out=idn, in_=idn_d[:, :])

            # ---- phase 1: build EP table ---------------------------------
            def _body():
              p1ps_cm = tc.tile_pool(name="p1ps", bufs=2, space="PSUM")
              p1ps = p1ps_cm.__enter__()
              fblk = 896 if KSHARD else FBLK
              nstrips = p1n // fblk if "1" in PHASES else 0
              for st in range(nstrips):
                  fstrip = fpool.tile([P, fblk], BF16)
                  nc.sync.dma_start(
                      out=fstrip, in_=fcm_d[:, st * fblk:(st + 1) * fblk]
                  )
                  for b in range(fblk // P):
                      frhs = fstrip[:, b * P:(b + 1) * P]
                      x_ps = p1ps.tile([P, 2, P], F32, tag="xps")
                      for m in range(2):
                          nc.tensor.matmul(
                              x_ps[:, m, :], lhsT=w1s[:, m * P:(m + 1) * P],
                              rhs=frhs, start=True, stop=True,
                          )
                      x_sb = p1w.tile([P, 2, P], BF16, tag="x")
                      for m in range(2):
                          nc.vector.tensor_scalar(
                              out=x_sb[:, m, :], in0=x_ps[:, m, :],
                              scalar1=b1c[:, m:m + 1], scalar2=0.0,
                              op0=mybir.AluOpType.add, op1=mybir.AluOpType.max,
                          )
                      l_ps = p1ps.tile([P, 2, P], F32, tag="lps")
                      for mp in range(2):
                          for k in range(2):
                              nc.tensor.matmul(
                                  l_ps[:, mp, :],
                                  lhsT=wss[:, k, mp * P:(mp + 1) * P],
                                  rhs=x_sb[:, k, :],
                                  start=(k == 0), stop=(k == 1),
                              )
                      e_sb = p1w.tile([P, 2, P], BF16, tag="e")
                      for mp in range(2):
                          nc.scalar.activation(
                              out=e_sb[:, mp, :], in_=l_ps[:, mp, :],
                              func=mybir.ActivationFunctionType.Exp,
                              bias=csc[:, mp:mp + 1], scale=1.0,
                          )
                      p_sb = p1w.tile([P, 2, P], BF16, tag="p")
                      nc.gpsimd.tensor_tensor(
                          out=p_sb[:, :, :].rearrange("p m c -> p (m c)"),
                          in0=e_sb[:, :, :].rearrange("p m c -> p (m c)"),
                          in1=x_sb[:, :, :].rearrange("p m c -> p (m c)"),
                          op=mybir.AluOpType.mult,
                      )
                      ep_ps = p1ps.tile([P, 2 * C2], BF16, tag="ep")
                      for m in range(2):
                          nc.tensor.transpose(
                              ep_ps[:, m * P:(m + 1) * P], e_sb[:, m, :], idn
                          )
                          nc.tensor.transpose(
                              ep_ps[:, C2 + m * P:C2 + (m + 1) * P], p_sb[:, m, :], idn
                          )
                      ep_sb = p1w.tile([P, 2 * C2], tdt, tag="eps")
                      nc.scalar.activation(
                          out=ep_sb[:, 0:C2], in_=ep_ps[:, 0:C2],
                          func=mybir.ActivationFunctionType.Copy,
                      )
                      nc.vector.tensor_copy(ep_sb[:, C2:2 * C2], ep_ps[:, C2:2 * C2])
                      nr = st * fblk + b * P
                      nc.sync.dma_start(out=tabs_d[nr:nr + P, :], in_=ep_sb)

              # ---- phase 2: gather + segment-matmul aggregate --------------
              if KSHARD and "1" in PHASES:
                  nc.gpsimd.collective_compute(
                      "AllGather", mybir.AluOpType.bypass,
                      replica_groups=[list(range(N_CORES))],
                      ins=[tabs_d.ap().opt()], outs=[tab_d.ap().opt()],
                  )
              p1ps_cm.__exit__(None, None, None)
              p2ps_cm = tc.tile_pool(name="p2ps", bufs=2, space="PSUM")
              p2ps = p2ps_cm.__enter__()
              idx_off = 0      # int16 columns of idx_d consumed so far
              chunk_off = 0    # chunks of s_d consumed so far
              sups = list(range(nsup)) if "2" in PHASES else []
              t_all = None
              if sups and not KNOTAIL:
                  t_all = tpool.tile([P, nsup, C2], BF16, tag="tall")
              idx_all = None
              if sups:
                  idx_all = tpool.tile([P, idx_tot // 16], mybir.dt.int16,
                                       tag="idxall")
                  nc.scalar.dma_start(out=idx_all, in_=idx_d[:, :])
              if not sups:
                  zz = opool.tile([P, C_OUT], F32)
                  nc.vector.memset(zz, 0.0)
                  for s in range(nsup):
                      nc.sync.dma_start(out=out_d[s * SUP:(s + 1) * SUP, :], in_=zz)
              for s in sups:
                  ssz = [sizes[t][s] for t in range(NSUB)]
                  stot = sum(ssz)
                  idx_t = idx_all[:, idx_off:idx_off + stot // 16]
                  g_t = gpool.tile([P, stot // P, 2 * C2], tdt, tag="g")
                  io = 0
                  for t in range(NSUB):
                      if ssz[t] == 0:
                          continue
                      if KDENSE:
                          nc.sync.dma_start(
                              out=g_t[:, io // P:(io + ssz[t]) // P, :]
                              .rearrange("p a b -> p (a b)"),
                              in_=bass.AP(
                                  tensor=tab_d,
                                  offset=0,
                                  ap=[[2 * C2, P], [P * 2 * C2, ssz[t] // P],
                                      [1, 2 * C2]],
                              ),
                          )
                      else:
                          nc.gpsimd.dma_gather(
                              g_t[:, io // P:(io + ssz[t]) // P, :],
                              tab_d[TBOUND[t]:TBOUND[t + 1], :],
                              idx_t[:, io // 16:(io + ssz[t]) // 16],
                              num_idxs=ssz[t], num_idxs_reg=ssz[t],
                              elem_size=2 * C2, transpose=False,
                              queue_num=t,
                          )
                      io += ssz[t]
                  nchunks = stot // P
                  s_t = spool.tile([P, nchunks, P], tdt, tag="s")
                  nc.scalar.dma_start(
                      out=s_t.rearrange("p a b -> p (a b)"),
                      in_=s_d[:, chunk_off * P:(chunk_off + nchunks) * P],
                  )
                  idx_off += stot // 16
                  chunk_off += nchunks

                  acc = p2ps.tile([P, 2 * C2], F32, tag="acc")
                  if DROW:
                      assert nchunks % 2 == 0
                      for ci in range(0, nchunks, 2):
                          nc.tensor.matmul(
                              acc,
                              lhsT=s_t[:, ci:ci + 2, :],
                              rhs=g_t[:, ci:ci + 2, :],
                              start=(ci == 0), stop=(ci == nchunks - 2),
                              perf_mode=mybir.MatmulPerfMode.DoubleRow,
                          )
                  else:
                      for ci in range(nchunks):
                          nc.tensor.matmul(
                              acc,
                              lhsT=s_t[:, ci, :],
                              rhs=g_t[:, ci, :],
                              start=(ci == 0), stop=(ci == nchunks - 1),
                          )
                  if KNOTAIL:
                      oo = opool.tile([P, C_OUT], F32)
                      nc.vector.tensor_copy(oo, acc[:, 0:C_OUT])
                      nc.scalar.dma_start(
                          out=out_d[s * SUP:(s + 1) * SUP, :], in_=oo
                      )
                      continue
                  rec = p2w.tile([P, C2], F32, tag="rec")
                  nc.vector.reciprocal_approx_fast(rec, acc[:, 0:C2])
                  nc.vector.tensor_tensor(
                      out=t_all[:, s, :], in0=acc[:, C2:2 * C2], in1=rec,
                      op=mybir.AluOpType.mult,
                  )
              p2ps_cm.__exit__(None, None, None)

              # ---- pass 2b: transpose t + final matmul, dense PE stream ----
              if sups and not KNOTAIL:
                  p3ps_cm = tc.tile_pool(name="p3ps", bufs=4, space="PSUM")
                  p3ps = p3ps_cm.__enter__()
                  for s in sups:
                      tT_ps = p3ps.tile([P, 2, P], BF16, tag="tT")
                      for k in range(2):
                          nc.tensor.transpose(
                              tT_ps[:, k, :], t_all[:, s, k * P:(k + 1) * P], idn
                          )
                      tT_sb = p2w.tile([P, 2, P], BF16, tag="tTs")
                      nc.scalar.activation(
                          out=tT_sb.rearrange("p a b -> p (a b)"),
                          in_=tT_ps.rearrange("p a b -> p (a b)"),
                          func=mybir.ActivationFunctionType.Copy,
                      )
                      o_ps = p3ps.tile([P, C_OUT], F32, tag="ops")
                      for k in range(2):
                          nc.tensor.matmul(
                              o_ps, lhsT=tT_sb[:, k, :], rhs=wms[:, k, :],
                              start=(k == 0), stop=(k == 1),
                          )
                      oo = opool.tile([P, C_OUT], F32)
                      nc.vector.tensor_tensor(
                          out=oo, in0=o_ps, in1=bmb, op=mybir.AluOpType.add,
                      )
                      nc.scalar.dma_start(
                          out=out_d[s * SUP:(s + 1) * SUP, :], in_=oo
                      )
                  p3ps_cm.__exit__(None, None, None)

            for _rep in range(KREPEAT):
                _body()

    nc.compile()
    return nc


def _get_prog(nsup, sizes):
    key = (nsup, tuple(map(tuple, sizes)), TAB_FP8, PHASES, TAB_KIND)
    if key not in _PROG_CACHE:
        _PROG_CACHE[key] = build_program(nsup, sizes)
    return _PROG_CACHE[key]


def _wrap16(flat):
    """int16 index layout for dma_gather: [16-partition wrap, replicated x8]."""
    n = flat.shape[0]
    w = flat.reshape(n // 16, 16).T              # [16, n/16]
    return np.tile(w, (8, 1))                    # [128, n/16]


def prep_inputs(features, neighbor_idx, W1, b1, gamma, beta, run_mean, run_var,
                Ws, Wm, bm, n_cores=N_CORES):
    bf16 = ml_dtypes.bfloat16
    f8 = ml_dtypes.float8_e4m3fn
    tdt_np = f8 if TAB_FP8 else bf16

    a = (gamma / np.sqrt(run_var + BN_EPS)).astype(np.float32)
    c = (beta - run_mean * a).astype(np.float32)
    wsp = (a[:, None] * Ws).astype(bf16)
    csv = (c.astype(np.float64) @ np.asarray(Ws, np.float64)).astype(np.float32)
    wmp = (a[:, None] * Wm).astype(bf16)
    bmv = (c.astype(np.float64) @ np.asarray(Wm, np.float64) + bm).astype(np.float32)

    fcm = np.zeros((P, NT), dtype=bf16)
    fcm[:, :N_NODES] = np.asarray(features, np.float32).T.astype(bf16)

    n_pc = neighbor_idx.shape[0] // n_cores
    nodes_pc = -(-n_pc // SUP) * SUP
    nsup = nodes_pc // SUP

    ni_all = np.asarray(neighbor_idx, np.int64)
    # core_edges[core][sup][sub] = (j_rel, n_local)
    core_edges = []
    cnt = np.zeros((NSUB, n_cores, nsup), np.int64)
    for ci in range(n_cores):
        ni = ni_all[ci * n_pc:(ci + 1) * n_pc]
        if nodes_pc != n_pc:
            ni = np.concatenate(
                [ni, np.zeros((nodes_pc - n_pc, K_NBR), np.int64)], axis=0)
        per_sup = []
        for s in range(nsup):
            e = ni[s * SUP:(s + 1) * SUP]
            nloc = np.repeat(np.arange(SUP), K_NBR)
            j = e.reshape(-1)
            subs = []
            for t in range(NSUB):
                m = (j >= TBOUND[t]) & (j < TBOUND[t + 1])
                jt, nt_ = j[m] - TBOUND[t], nloc[m]
                o = np.argsort(jt, kind="stable")   # row-sorted for HBM locality
                subs.append((jt[o], nt_[o]))
                cnt[t, ci, s] = m.sum()
            per_sup.append(subs)
        core_edges.append(per_sup)

    sizes = [
        [int(-(-cnt[t, :, s].max() // P) * P) for s in range(nsup)]
        for t in range(NSUB)
    ]
    for s in range(nsup):
        if sum(sizes[t][s] for t in range(NSUB)) % 256 != 0:
            tmin = min(range(NSUB), key=lambda t: sizes[t][s])
            sizes[tmin][s] += P          # keep chunk count even for DoubleRow
    for t in range(NSUB):
        for s in range(nsup):
            assert sizes[t][s] <= GCAP, (t, s, sizes[t][s])

    tot_chunks = sum(sizes[t][s] for t in range(NSUB) for s in range(nsup)) // P
    idx_tot = tot_chunks * P

    shared = dict(
        fcm=fcm, w1=np.ascontiguousarray(W1.astype(bf16)),
        wsp=np.ascontiguousarray(wsp), wmp=np.ascontiguousarray(wmp),
        b1c=np.ascontiguousarray(b1.astype(np.float32).reshape(2, P).T),
        csc=np.ascontiguousarray(csv.reshape(2, P).T),
        bmr=bmv.reshape(1, C_OUT),
        idn=np.eye(P, dtype=bf16),
    )

    in_maps = []
    for ci in range(n_cores):
        if KSHARD:
            shared = dict(shared, fcm=np.ascontiguousarray(
                fcm[:, ci * NTS:(ci + 1) * NTS]))
        idx_cols = []
        s_mat = np.zeros((tot_chunks * P, P), np.uint8)
        chunk_base = 0
        for s in range(nsup):
            for t in range(NSUB):
                jl, nl = core_edges[ci][s][t]
                size = sizes[t][s]
                idx = np.zeros(size, np.int16)
                idx[:len(jl)] = jl.astype(np.int16)
                idx_cols.append(idx)
                base = chunk_base * P
                s_mat[base + np.arange(len(nl)), nl] = 1
                chunk_base += size // P
        idx_flat = np.concatenate(idx_cols)
        assert idx_flat.shape[0] == idx_tot
        idxs = np.ascontiguousarray(_wrap16(idx_flat))
        s_re = s_mat.reshape(tot_chunks, P, P).transpose(1, 0, 2).reshape(
            P, tot_chunks * P)
        smat = s_re.astype(np.float32).astype(tdt_np)
        in_maps.append(dict(shared, idxs=idxs, smat=np.ascontiguousarray(smat)))

    return in_maps, nsup, sizes, n_pc


def kernel(**inputs):
    in_maps, nsup, sizes, n_pc = prep_inputs(**inputs)
    nc = _get_prog(nsup, sizes)
    res = run_bass_kernel_spmd(nc, in_maps, core_ids=list(range(N_CORES)))
    return np.concatenate([r["out"][:n_pc] for r in res.results], axis=0)



# revision 3
# speedup vs baseline: 1.0997x; 1.0997x over previous
"""LocalFeatureAggregation Trainium2 kernel (8 NeuronCores, data-parallel over nodes).

Architecture ("logit-table + segment-matmul"):

  Math identity: with BN folded (g = a*x + c), the attention logits
  L[n,k,:] = g[n,k,:] @ Ws depend only on the SOURCE node j = idx[n,k]:
      L[j] = x[j] @ (a*Ws) + c@Ws
      feat[n] = a * (sum_k P[j]) / (sum_k E[j]) + c,  E = exp(L), P = E*x
      out[n]  = (sum_k P / sum_k E) @ (a*Wm) + (c@Wm + bm)
  so all exp/matmul work is PER NODE (50k) instead of PER EDGE (800k).

  Phase 1 (replicated on each core, channel-major):
      x = relu(f@W1 + b1)   [DVE evac, bias fused]
      L = x @ Ws' (+cs bias fused into exp)  -> E = exp(L)  [Act]
      P = E * x  [Pool]
      4 PE transposes assemble node-major rows [E(256) | P(256)], evac to
      SBUF as fp8 -> DRAM table (fp8 e4m3: validated |L|<3, P<50, no overflow).

  Phase 2 (per core, 49 sups of 128 destination nodes):
      dma_gather (transpose=False) fetches each edge's 512B table row onto
      partition slot%128. int16 index limit + the HW ~1024-idx/call ring cap
      are handled by THREE subtables (rows [0,16768), [16768,33536),
      [33536,50176)) with per-sup compact slot lists padded to 128 multiples
      (pad idx=0, killed by zero columns in S).
      Aggregation sum_k via one-hot segment matmuls: lhsT = S[slot, node]
      (host-built 0/1), rhs = gathered rows -> PSUM [128 nodes, 512 f32].
      Then t = P_sum * recip(E_sum) [DVE], PE-transpose t, final matmul with
      wmp = a*Wm, bias bmp = c@Wm + bm added on DVE, DMA out.
"""

import os

import numpy as np
import ml_dtypes

import concourse.bass as bass
import concourse.bacc as bacc
import concourse.tile as tile
from concourse import mybir
from concourse.bass_utils import run_bass_kernel_spmd

BN_EPS = 1e-5
P = 128
N_NODES = 50000
K_NBR = 16
C_IN = 128
C2 = 256
C_OUT = 128
N_CORES = 8
NODES_PC_RAW = N_NODES // N_CORES            # 6250
NT = 50176                                   # padded table rows (392*128)
TBOUND = (0, 16768, 33536, NT)               # 3 subtables, each < 32768 rows
NSUB = 3
SUP = 128                                    # dest nodes per sup
EPS = 2048                                   # edges per sup
FBLK = 512                                   # phase-1 feature strip columns (non-shard)
GCAP = 1024                                  # HW cap on idxs per dma_gather

BF16 = mybir.dt.bfloat16
F32 = mybir.dt.float32
FP8 = mybir.dt.float8e4

TAB_FP8 = True                               # table dtype switch
PHASES = os.environ.get("KPHASES", "12")
TAB_KIND = os.environ.get("KTABKIND", "Internal")
KREPEAT = int(os.environ.get("KREPEAT", "1"))  # loop body N times (timing only)
DROW = os.environ.get("KDROW", "1") == "1" and TAB_FP8  # DoubleRow fp8 agg
KSHARD = os.environ.get("KSHARD", "0") == "1"  # shard phase1 + AllGather
NTS = NT // N_CORES                            # 6272 table rows per core shard
KDENSE = os.environ.get("KDENSE", "0") == "1"  # timing: dense DMA replaces gathers
KNOTAIL = os.environ.get("KNOTAIL", "0") == "1"  # timing: skip divide/final tail
KQUEUES = int(os.environ.get("KQUEUES", "3"))    # SWDGE queue count (1-4)
KSCRATCH = int(os.environ.get("KSCRATCH", "16384"))  # SWDGE desc scratch bytes

_PROG_CACHE = {}


def build_program(nsup, sizes):
    """sizes: [NSUB][nsup] padded slot counts (multiples of 128, <= GCAP)."""
    tdt = FP8 if TAB_FP8 else BF16
    nodes_pc = nsup * SUP
    nc = bacc.Bacc("TRN2", num_devices=N_CORES, num_swdge_queues=KQUEUES,
                   dynamic_dma_scratch_size=KSCRATCH)

    p1n = NTS if KSHARD else NT              # phase-1 nodes computed per core
    fcm_d = nc.dram_tensor("fcm", [P, p1n], BF16, kind="ExternalInput")
    w1_d = nc.dram_tensor("w1", [C_IN, C2], BF16, kind="ExternalInput")
    ws_d = nc.dram_tensor("wsp", [C2, C2], BF16, kind="ExternalInput")
    wm_d = nc.dram_tensor("wmp", [C2, C_OUT], BF16, kind="ExternalInput")
    b1_d = nc.dram_tensor("b1c", [P, 2], F32, kind="ExternalInput")
    cs_d = nc.dram_tensor("csc", [P, 2], F32, kind="ExternalInput")
    bm_d = nc.dram_tensor("bmr", [1, C_OUT], F32, kind="ExternalInput")
    idn_d = nc.dram_tensor("idn", [P, P], BF16, kind="ExternalInput")

    tot_chunks = sum(sizes[t][s] for t in range(NSUB) for s in range(nsup)) // P
    idx_tot = tot_chunks * P
    idx_d = nc.dram_tensor("idxs", [P, idx_tot // 16], mybir.dt.int16,
                           kind="ExternalInput")
    s_d = nc.dram_tensor("smat", [P, tot_chunks * P], tdt, kind="ExternalInput")

    if TAB_KIND == "Internal":
        tab_d = nc.dram_tensor("tab", [NT, 2 * C2], tdt, kind="Internal")
    elif TAB_KIND == "ExternalOutput":
        tab_d = nc.dram_tensor("tab", [NT, 2 * C2], tdt, kind="ExternalOutput")
    else:
        tab_d = nc.dram_tensor("tab", [NT, 2 * C2], tdt, kind="ExternalInput")
    tabs_d = (nc.dram_tensor("tabs", [NTS, 2 * C2], tdt, kind="Internal")
              if KSHARD else tab_d)
    out_d = nc.dram_tensor("out", [nodes_pc, C_OUT], F32, kind="ExternalOutput")

    with tile.TileContext(nc) as tc:
        with (
            tc.tile_pool(name="consts", bufs=1) as consts,
            tc.tile_pool(name="fpool", bufs=2) as fpool,
            tc.tile_pool(name="p1w", bufs=3) as p1w,
            tc.tile_pool(name="gpool", bufs=4) as gpool,
            tc.tile_pool(name="spool", bufs=2) as spool,
            tc.tile_pool(name="p2w", bufs=3) as p2w,
            tc.tile_pool(name="tpool", bufs=1) as tpool,
            tc.tile_pool(name="opool", bufs=3) as opool,
        ):
            # ---- constants -----------------------------------------------
            w1s = consts.tile([P, C2], BF16)
            nc.sync.dma_start(out=w1s, in_=w1_d[:, :])
            wss = consts.tile([P, 2, C2], BF16)       # [k-chunk][ch', free]
            for k in range(2):
                nc.sync.dma_start(out=wss[:, k, :], in_=ws_d[k * P:(k + 1) * P, :])
            wms = consts.tile([P, 2, C_OUT], BF16)
            for k in range(2):
                nc.sync.dma_start(out=wms[:, k, :], in_=wm_d[k * P:(k + 1) * P, :])
            b1c = consts.tile([P, 2], F32)
            nc.sync.dma_start(out=b1c, in_=b1_d[:, :])
            csc = consts.tile([P, 2], F32)
            nc.sync.dma_start(out=csc, in_=cs_d[:, :])
            bmb = consts.tile([P, C_OUT], F32)
            nc.sync.dma_start(
                out=bmb, in_=bass.AP(tensor=bm_d, offset=0, ap=[[0, P], [1, C_OUT]])
            )
            idn = consts.tile([P, P], BF16)
            nc.sync.dma_start(out=idn, in_=idn_d[:, :])

            # ---- phase 1: build EP table ---------------------------------
            def _body():
              p1ps_cm = tc.tile_pool(name="p1ps", bufs=2, space="PSUM")
              p1ps = p1ps_cm.__enter__()
              fblk = 896 if KSHARD else FBLK
              nstrips = p1n // fblk if "1" in PHASES else 0
              for st in range(nstrips):
                  fstrip = fpool.tile([P, fblk], BF16)
                  nc.sync.dma_start(
                      out=fstrip, in_=fcm_d[:, st * fblk:(st + 1) * fblk]
                  )
                  for b in range(fblk // P):
                      frhs = fstrip[:, b * P:(b + 1) * P]
                      x_ps = p1ps.tile([P, 2, P], F32, tag="xps")
                      for m in range(2):
                          nc.tensor.matmul(
                              x_ps[:, m, :], lhsT=w1s[:, m * P:(m + 1) * P],
                              rhs=frhs, start=True, stop=True,
                          )
                      x_sb = p1w.tile([P, 2, P], BF16, tag="x")
                      for m in range(2):
                          nc.vector.tensor_scalar(
                              out=x_sb[:, m, :], in0=x_ps[:, m, :],
                              scalar1=b1c[:, m:m + 1], scalar2=0.0,
                              op0=mybir.AluOpType.add, op1=mybir.AluOpType.max,
                          )
                      l_ps = p1ps.tile([P, 2, P], F32, tag="lps")
                      for mp in range(2):
                          for k in range(2):
                              nc.tensor.matmul(
                                  l_ps[:, mp, :],
                                  lhsT=wss[:, k, mp * P:(mp + 1) * P],
                                  rhs=x_sb[:, k, :],
                                  start=(k == 0), stop=(k == 1),
                              )
                      e_sb = p1w.tile([P, 2, P], BF16, tag="e")
                      for mp in range(2):
                          nc.scalar.activation(
                              out=e_sb[:, mp, :], in_=l_ps[:, mp, :],
                              func=mybir.ActivationFunctionType.Exp,
                              bias=csc[:, mp:mp + 1], scale=1.0,
                          )
                      p_sb = p1w.tile([P, 2, P], BF16, tag="p")
                      nc.gpsimd.tensor_tensor(
                          out=p_sb[:, :, :].rearrange("p m c -> p (m c)"),
                          in0=e_sb[:, :, :].rearrange("p m c -> p (m c)"),
                          in1=x_sb[:, :, :].rearrange("p m c -> p (m c)"),
                          op=mybir.AluOpType.mult,
                      )
                      ep_ps = p1ps.tile([P, 2 * C2], BF16, tag="ep")
                      for m in range(2):
                          nc.tensor.transpose(
                              ep_ps[:, m * P:(m + 1) * P], e_sb[:, m, :], idn
                          )
                          nc.tensor.transpose(
                              ep_ps[:, C2 + m * P:C2 + (m + 1) * P], p_sb[:, m, :], idn
                          )
                      ep_sb = p1w.tile([P, 2 * C2], tdt, tag="eps")
                      nc.scalar.activation(
                          out=ep_sb[:, 0:C2], in_=ep_ps[:, 0:C2],
                          func=mybir.ActivationFunctionType.Copy,
                      )
                      nc.vector.tensor_copy(ep_sb[:, C2:2 * C2], ep_ps[:, C2:2 * C2])
                      nr = st * fblk + b * P
                      nc.sync.dma_start(out=tabs_d[nr:nr + P, :], in_=ep_sb)

              # ---- phase 2: gather + segment-matmul aggregate --------------
              if KSHARD and "1" in PHASES:
                  nc.gpsimd.collective_compute(
                      "AllGather", mybir.AluOpType.bypass,
                      replica_groups=[list(range(N_CORES))],
                      ins=[tabs_d.ap().opt()], outs=[tab_d.ap().opt()],
                  )
              p1ps_cm.__exit__(None, None, None)
              p2ps_cm = tc.tile_pool(name="p2ps", bufs=2, space="PSUM")
              p2ps = p2ps_cm.__enter__()
              idx_off = 0      # int16 columns of idx_d consumed so far
              chunk_off = 0    # chunks of s_d consumed so far
              sups = list(range(nsup)) if "2" in PHASES else []
              t_all = None
              if sups and not KNOTAIL:
                  t_all = tpool.tile([P, nsup, C2], BF16, tag="tall")
              idx_all = None
              if sups:
                  idx_all = tpool.tile([P, idx_tot // 16], mybir.dt.int16,
                                       tag="idxall")
                  nc.scalar.dma_start(out=idx_all, in_=idx_d[:, :])
              if not sups:
                  zz = opool.tile([P, C_OUT], F32)
                  nc.vector.memset(zz, 0.0)
                  for s in range(nsup):
                      nc.sync.dma_start(out=out_d[s * SUP:(s + 1) * SUP, :], in_=zz)
              for s in sups:
                  ssz = [sizes[t][s] for t in range(NSUB)]
                  stot = sum(ssz)
                  idx_t = idx_all[:, idx_off:idx_off + stot // 16]
                  g_t = gpool.tile([P, stot // P, 2 * C2], tdt, tag="g")
                  io = 0
                  for t in range(NSUB):
                      if ssz[t] == 0:
                          continue
                      if KDENSE:
                          nc.sync.dma_start(
                              out=g_t[:, io // P:(io + ssz[t]) // P, :]
                              .rearrange("p a b -> p (a b)"),
                              in_=bass.AP(
                                  tensor=tab_d,
                                  offset=0,
                                  ap=[[2 * C2, P], [P * 2 * C2, ssz[t] // P],
                                      [1, 2 * C2]],
                              ),
                          )
                      else:
                          nc.gpsimd.dma_gather(
                              g_t[:, io // P:(io + ssz[t]) // P, :],
                              tab_d[TBOUND[t]:TBOUND[t + 1], :],
                              idx_t[:, io // 16:(io + ssz[t]) // 16],
                              num_idxs=ssz[t], num_idxs_reg=ssz[t],
                              elem_size=2 * C2, transpose=False,
                              queue_num=(s * NSUB + t) % KQUEUES,
                          )
                      io += ssz[t]
                  nchunks = stot // P
                  s_t = spool.tile([P, nchunks, P], tdt, tag="s")
                  nc.scalar.dma_start(
                      out=s_t.rearrange("p a b -> p (a b)"),
                      in_=s_d[:, chunk_off * P:(chunk_off + nchunks) * P],
                  )
                  idx_off += stot // 16
                  chunk_off += nchunks

                  acc = p2ps.tile([P, 2 * C2], F32, tag="acc")
                  if DROW:
                      assert nchunks % 2 == 0
                      for ci in range(0, nchunks, 2):
                          nc.tensor.matmul(
                              acc,
                              lhsT=s_t[:, ci:ci + 2, :],
                              rhs=g_t[:, ci:ci + 2, :],
                              start=(ci == 0), stop=(ci == nchunks - 2),
                              perf_mode=mybir.MatmulPerfMode.DoubleRow,
                          )
                  else:
                      for ci in range(nchunks):
                          nc.tensor.matmul(
                              acc,
                              lhsT=s_t[:, ci, :],
                              rhs=g_t[:, ci, :],
                              start=(ci == 0), stop=(ci == nchunks - 1),
                          )
                  if KNOTAIL:
                      oo = opool.tile([P, C_OUT], F32)
                      nc.vector.tensor_copy(oo, acc[:, 0:C_OUT])
                      nc.scalar.dma_start(
                          out=out_d[s * SUP:(s + 1) * SUP, :], in_=oo
                      )
                      continue
                  rec = p2w.tile([P, C2], F32, tag="rec")
                  nc.vector.reciprocal_approx_fast(rec, acc[:, 0:C2])
                  nc.vector.tensor_tensor(
                      out=t_all[:, s, :], in0=acc[:, C2:2 * C2], in1=rec,
                      op=mybir.AluOpType.mult,
                  )
              p2ps_cm.__exit__(None, None, None)

              # ---- pass 2b: transpose t + final matmul, dense PE stream ----
              if sups and not KNOTAIL:
                  p3ps_cm = tc.tile_pool(name="p3ps", bufs=4, space="PSUM")
                  p3ps = p3ps_cm.__enter__()
                  for s in sups:
                      tT_ps = p3ps.tile([P, 2, P], BF16, tag="tT")
                      for k in range(2):
                          nc.tensor.transpose(
                              tT_ps[:, k, :], t_all[:, s, k * P:(k + 1) * P], idn
                          )
                      tT_sb = p2w.tile([P, 2, P], BF16, tag="tTs")
                      nc.scalar.activation(
                          out=tT_sb.rearrange("p a b -> p (a b)"),
                          in_=tT_ps.rearrange("p a b -> p (a b)"),
                          func=mybir.ActivationFunctionType.Copy,
                      )
                      o_ps = p3ps.tile([P, C_OUT], F32, tag="ops")
                      for k in range(2):
                          nc.tensor.matmul(
                              o_ps, lhsT=tT_sb[:, k, :], rhs=wms[:, k, :],
                              start=(k == 0), stop=(k == 1),
                          )
                      oo = opool.tile([P, C_OUT], F32)
                      nc.vector.tensor_tensor(
                          out=oo, in0=o_ps, in1=bmb, op=mybir.AluOpType.add,
                      )
                      nc.scalar.dma_start(
                          out=out_d[s * SUP:(s + 1) * SUP, :], in_=oo
                      )
                  p3ps_cm.__exit__(None, None, None)

            for _rep in range(KREPEAT):
                _body()

    nc.compile()
    return nc


def _get_prog(nsup, sizes):
    key = (nsup, tuple(map(tuple, sizes)), TAB_FP8, PHASES, TAB_KIND, KQUEUES, KSCRATCH)
    if key not in _PROG_CACHE:
        _PROG_CACHE[key] = build_program(nsup, sizes)
    return _PROG_CACHE[key]


def _wrap16(flat):
    """int16 index layout for dma_gather: [16-partition wrap, replicated x8]."""
    n = flat.shape[0]
    w = flat.reshape(n // 16, 16).T              # [16, n/16]
    return np.tile(w, (8, 1))                    # [128, n/16]


def prep_inputs(features, neighbor_idx, W1, b1, gamma, beta, run_mean, run_var,
                Ws, Wm, bm, n_cores=N_CORES):
    bf16 = ml_dtypes.bfloat16
    f8 = ml_dtypes.float8_e4m3fn
    tdt_np = f8 if TAB_FP8 else bf16

    a = (gamma / np.sqrt(run_var + BN_EPS)).astype(np.float32)
    c = (beta - run_mean * a).astype(np.float32)
    wsp = (a[:, None] * Ws).astype(bf16)
    csv = (c.astype(np.float64) @ np.asarray(Ws, np.float64)).astype(np.float32)
    wmp = (a[:, None] * Wm).astype(bf16)
    bmv = (c.astype(np.float64) @ np.asarray(Wm, np.float64) + bm).astype(np.float32)

    fcm = np.zeros((P, NT), dtype=bf16)
    fcm[:, :N_NODES] = np.asarray(features, np.float32).T.astype(bf16)

    n_pc = neighbor_idx.shape[0] // n_cores
    nodes_pc = -(-n_pc // SUP) * SUP
    nsup = nodes_pc // SUP

    ni_all = np.asarray(neighbor_idx, np.int64)
    # core_edges[core][sup][sub] = (j_rel, n_local)
    core_edges = []
    cnt = np.zeros((NSUB, n_cores, nsup), np.int64)
    for ci in range(n_cores):
        ni = ni_all[ci * n_pc:(ci + 1) * n_pc]
        if nodes_pc != n_pc:
            ni = np.concatenate(
                [ni, np.zeros((nodes_pc - n_pc, K_NBR), np.int64)], axis=0)
        per_sup = []
        for s in range(nsup):
            e = ni[s * SUP:(s + 1) * SUP]
            nloc = np.repeat(np.arange(SUP), K_NBR)
            j = e.reshape(-1)
            subs = []
            for t in range(NSUB):
                m = (j >= TBOUND[t]) & (j < TBOUND[t + 1])
                jt, nt_ = j[m] - TBOUND[t], nloc[m]
                o = np.argsort(jt, kind="stable")   # row-sorted for HBM locality
                subs.append((jt[o], nt_[o]))
                cnt[t, ci, s] = m.sum()
            per_sup.append(subs)
        core_edges.append(per_sup)

    sizes = [
        [int(-(-cnt[t, :, s].max() // P) * P) for s in range(nsup)]
        for t in range(NSUB)
    ]
    for s in range(nsup):
        if sum(sizes[t][s] for t in range(NSUB)) % 256 != 0:
            tmin = min(range(NSUB), key=lambda t: sizes[t][s])
            sizes[tmin][s] += P          # keep chunk count even for DoubleRow
    for t in range(NSUB):
        for s in range(nsup):
            assert sizes[t][s] <= GCAP, (t, s, sizes[t][s])

    tot_chunks = sum(sizes[t][s] for t in range(NSUB) for s in range(nsup)) // P
    idx_tot = tot_chunks * P

    shared = dict(
        fcm=fcm, w1=np.ascontiguousarray(W1.astype(bf16)),
        wsp=np.ascontiguousarray(wsp), wmp=np.ascontiguousarray(wmp),
        b1c=np.ascontiguousarray(b1.astype(np.float32).reshape(2, P).T),
        csc=np.ascontiguousarray(csv.reshape(2, P).T),
        bmr=bmv.reshape(1, C_OUT),
        idn=np.eye(P, dtype=bf16),
    )

    in_maps = []
    for ci in range(n_cores):
        if KSHARD:
            shared = dict(shared, fcm=np.ascontiguousarray(
                fcm[:, ci * NTS:(ci + 1) * NTS]))
        idx_cols = []
        s_mat = np.zeros((tot_chunks * P, P), np.uint8)
        chunk_base = 0
        for s in range(nsup):
            for t in range(NSUB):
                jl, nl = core_edges[ci][s][t]
                size = sizes[t][s]
                idx = np.zeros(size, np.int16)
                idx[:len(jl)] = jl.astype(np.int16)
                idx_cols.append(idx)
                base = chunk_base * P
                s_mat[base + np.arange(len(nl)), nl] = 1
                chunk_base += size // P
        idx_flat = np.concatenate(idx_cols)
        assert idx_flat.shape[0] == idx_tot
        idxs = np.ascontiguousarray(_wrap16(idx_flat))
        s_re = s_mat.reshape(tot_chunks, P, P).transpose(1, 0, 2).reshape(
            P, tot_chunks * P)
        smat = s_re.astype(np.float32).astype(tdt_np)
        in_maps.append(dict(shared, idxs=idxs, smat=np.ascontiguousarray(smat)))

    return in_maps, nsup, sizes, n_pc


def kernel(**inputs):
    in_maps, nsup, sizes, n_pc = prep_inputs(**inputs)
    nc = _get_prog(nsup, sizes)
    res = run_bass_kernel_spmd(nc, in_maps, core_ids=list(range(N_CORES)))
    return np.concatenate([r["out"][:n_pc] for r in res.results], axis=0)



# revision 5
# speedup vs baseline: 1.3807x; 1.2555x over previous
"""LocalFeatureAggregation Trainium2 kernel (8 NeuronCores, data-parallel over nodes).

v2 architecture ("node-major logit table + round-interleaved gather"):

  Math identity: with BN folded (g = a*x + c), per-channel softmax over the
  k neighbors depends only on source j = idx[n,k]:
      L[j] = x[j] @ (a*Ws)          (c@Ws shift dropped: softmax-invariant)
      E = exp(L), P = E * x
      feat[n] = a * (sum_k P[j]) / (sum_k E[j]) + c
      out[n]  = (sum_k P / sum_k E) @ (a*Wm) + (c@Wm + bm)

  Phase 1 (replicated per core, 196 half-strips of 256 nodes, NODE-major):
      x_cm = relu(W1^T-mm + b1)            [PE 2mm, DVE evac]
      x_nm = relu(fcm-block-mm @ W1 + b1)  [PE 2mm + ones-row bias mm, PSUM]
      L_nm = x_cm-chunks-mm @ Ws'          [PE 4mm, node-major PSUM]
      E = exp(L) -> ep_sb[:, :, 0:256] fp8     [Act evac]
      P = (x_nm max 0) * E -> ep_sb[:, :, 256:512] fp8  [DVE fused]
      one DMA per half-strip writes 256 table rows [E|P] (512B, fp8).

  Phase 2 interleaved by ROUNDS over 4 equal subtables (12544 rows each,
  int16-safe, 49 half-strips each): after subtable t is built, its 49
  per-sup gathers (dma_gather, 512B rows, ~512 idx/call, 4 SWDGE queues)
  run while subtable t+1 builds. Segment one-hot matmuls (fp8 DoubleRow
  pairs + odd single) accumulate [sumE|sumP] in PSUM; rounds 0-2 add into
  an SBUF bf16 accumulator; round 3 fuses the tail per sup:
  sum -> recip(E) -> t = P/E -> SBUF DMA-transpose -> final matmul with
  wmp = a*Wm -> +(c@Wm + bm) -> out. GpSimd runs ONLY gathers.
"""

import os

import numpy as np
import ml_dtypes

import concourse.bass as bass
import concourse.bacc as bacc
import concourse.tile as tile
from concourse import mybir
from concourse.bass_utils import run_bass_kernel_spmd

BN_EPS = 1e-5
P = 128
N_NODES = 50000
K_NBR = 16
C_IN = 128
C2 = 256
C_OUT = 128
N_CORES = 8
NT = 50176                                   # padded table rows (392*128)
TBOUND = (0, 12544, 25088, 37632, NT)        # 4 equal subtables, 49 hs each
NSUB = 4
SUP = 128                                    # dest nodes per sup
HS = 256                                     # nodes per phase-1 half-strip
GCAP = 1024                                  # cap on idxs per dma_gather
KQUEUES = int(os.environ.get("KQUEUES", "4"))
KSCRATCH = int(os.environ.get("KSCRATCH", "65536"))
GBUFS = int(os.environ.get("KGBUFS", "6"))   # gather tile double-buffering

BF16 = mybir.dt.bfloat16
F32 = mybir.dt.float32
FP8 = mybir.dt.float8e4

_PROG_CACHE = {}


def build_program(nsup, sizes):
    """sizes: [NSUB][nsup] padded slot counts (multiples of 128, <= GCAP)."""
    nodes_pc = nsup * SUP
    nc = bacc.Bacc("TRN2", num_devices=N_CORES, num_swdge_queues=KQUEUES,
                   dynamic_dma_scratch_size=KSCRATCH)

    fcm_d = nc.dram_tensor("fcm", [P, NT], BF16, kind="ExternalInput")
    w1_d = nc.dram_tensor("w1", [C_IN, C2], BF16, kind="ExternalInput")
    ws_d = nc.dram_tensor("wsp", [C2, C2], BF16, kind="ExternalInput")
    wm_d = nc.dram_tensor("wmp", [C2, C_OUT], BF16, kind="ExternalInput")
    b1_d = nc.dram_tensor("b1c", [P, 2], F32, kind="ExternalInput")
    b1r_d = nc.dram_tensor("b1r", [1, C2], BF16, kind="ExternalInput")
    bm_d = nc.dram_tensor("bmr", [1, C_OUT], F32, kind="ExternalInput")

    tot_chunks = sum(sizes[t][s] for t in range(NSUB) for s in range(nsup)) // P
    idx_tot = tot_chunks * P
    idx_d = nc.dram_tensor("idxs", [P, idx_tot // 16], mybir.dt.int16,
                           kind="ExternalInput")
    s_d = nc.dram_tensor("smat", [P, tot_chunks * P], FP8, kind="ExternalInput")

    tab_d = nc.dram_tensor("tab", [NT, 2 * C2], FP8, kind="Internal")
    out_d = nc.dram_tensor("out", [nodes_pc, C_OUT], F32, kind="ExternalOutput")

    n_hs = [(TBOUND[t + 1] - TBOUND[t]) // HS for t in range(NSUB)]  # 49 each

    with tile.TileContext(nc) as tc:
        with (
            tc.tile_pool(name="consts", bufs=1) as consts,
            tc.tile_pool(name="fpool", bufs=2) as fpool,
            tc.tile_pool(name="p1w", bufs=2) as p1w,
            tc.tile_pool(name="gpool", bufs=GBUFS) as gpool,
            tc.tile_pool(name="spool", bufs=3) as spool,
            tc.tile_pool(name="p2w", bufs=2) as p2w,
            tc.tile_pool(name="tpool", bufs=1) as tpool,
            tc.tile_pool(name="opool", bufs=3) as opool,
            tc.tile_pool(name="xps", bufs=2, space="PSUM") as xps_p,
            tc.tile_pool(name="xnm", bufs=2, space="PSUM") as xnm_p,
            tc.tile_pool(name="lps", bufs=2, space="PSUM") as lps_p,
            tc.tile_pool(name="accp", bufs=1, space="PSUM") as acc_p,
            tc.tile_pool(name="opsp", bufs=1, space="PSUM") as ops_p,
        ):
            # ---- constants -----------------------------------------------
            w1s = consts.tile([P, C2], BF16)
            nc.sync.dma_start(out=w1s, in_=w1_d[:, :])
            wss = consts.tile([P, 2, C2], BF16)       # [k-chunk][ch', free]
            for k in range(2):
                nc.sync.dma_start(out=wss[:, k, :], in_=ws_d[k * P:(k + 1) * P, :])
            wms = consts.tile([P, 2, C_OUT], BF16)
            for k in range(2):
                nc.sync.dma_start(out=wms[:, k, :], in_=wm_d[k * P:(k + 1) * P, :])
            b1c = consts.tile([P, 2], F32)
            nc.sync.dma_start(out=b1c, in_=b1_d[:, :])
            b1r = consts.tile([1, C2], BF16)
            nc.sync.dma_start(out=b1r, in_=b1r_d[:, :])
            bmb = consts.tile([P, C_OUT], F32)
            nc.sync.dma_start(
                out=bmb, in_=bass.AP(tensor=bm_d, offset=0, ap=[[0, P], [1, C_OUT]])
            )
            ones = consts.tile([1, P], BF16)
            nc.vector.memset(ones, 1.0)
            idx_all = tpool.tile([P, idx_tot // 16], mybir.dt.int16, tag="idx")
            nc.scalar.dma_start(out=idx_all, in_=idx_d[:, :])
            acc_sb = tpool.tile([P, nsup, 2 * C2], BF16, tag="acc")

            # ---- per-half-strip phase-1 build ----------------------------
            def build_hs(nr):
                """nodes [nr, nr+HS): table rows via node-major compute."""
                fstrip = fpool.tile([P, HS], BF16)
                nc.sync.dma_start(out=fstrip, in_=fcm_d[:, nr:nr + HS])
                # x channel-major
                x_ps = xps_p.tile([P, 2, HS], F32, tag="xps")
                for h in range(2):
                    nc.tensor.matmul(
                        x_ps[:, h, :], lhsT=w1s[:, h * P:(h + 1) * P],
                        rhs=fstrip, start=True, stop=True,
                    )
                x_sb = p1w.tile([P, 2, HS], BF16, tag="x")
                for h in range(2):
                    nc.vector.tensor_scalar(
                        out=x_sb[:, h, :], in0=x_ps[:, h, :],
                        scalar1=b1c[:, h:h + 1], scalar2=0.0,
                        op0=mybir.AluOpType.add, op1=mybir.AluOpType.max,
                    )
                # x node-major (bias via ones-row mm; relu fused into P below)
                xn_ps = xnm_p.tile([P, 2, C2], F32, tag="xnm")
                for b in range(2):
                    nc.tensor.matmul(
                        xn_ps[:, b, :], lhsT=fstrip[:, b * P:(b + 1) * P],
                        rhs=w1s[:, :], start=True, stop=False,
                    )
                    nc.tensor.matmul(
                        xn_ps[:, b, :], lhsT=ones, rhs=b1r,
                        start=False, stop=True,
                    )
                # L node-major
                l_ps = lps_p.tile([P, 2, C2], F32, tag="lps")
                for b in range(2):
                    for h in range(2):
                        nc.tensor.matmul(
                            l_ps[:, b, :],
                            lhsT=x_sb[:, h, b * P:(b + 1) * P],
                            rhs=wss[:, h, :],
                            start=(h == 0), stop=(h == 1),
                        )
                ep_sb = p1w.tile([P, 2, 2 * C2], FP8, tag="ep")
                nc.scalar.activation(
                    out=ep_sb[:, :, 0:C2], in_=l_ps,
                    func=mybir.ActivationFunctionType.Exp, scale=1.0,
                )
                nc.vector.scalar_tensor_tensor(
                    out=ep_sb[:, :, C2:2 * C2], in0=xn_ps, scalar=0.0,
                    in1=ep_sb[:, :, 0:C2],
                    op0=mybir.AluOpType.max, op1=mybir.AluOpType.mult,
                )
                nc.sync.dma_start(
                    out=bass.AP(tensor=tab_d, offset=nr * 2 * C2,
                                ap=[[2 * C2, P], [P * 2 * C2, 2], [1, 2 * C2]]),
                    in_=ep_sb,
                )

            # ---- per-sup round processing --------------------------------
            idx_off = [0]
            chunk_off = [0]

            def process_sup(t, s):
                ssz = sizes[t][s]
                nch = ssz // P
                g_t = gpool.tile([P, nch, 2 * C2], FP8, tag="g")
                nc.gpsimd.dma_gather(
                    g_t, tab_d[TBOUND[t]:TBOUND[t + 1], :],
                    idx_all[:, idx_off[0]:idx_off[0] + ssz // 16],
                    num_idxs=ssz, num_idxs_reg=ssz,
                    elem_size=2 * C2, transpose=False,
                    queue_num=(t * nsup + s) % KQUEUES,
                )
                s_t = spool.tile([P, nch, P], FP8, tag="s")
                nc.scalar.dma_start(
                    out=s_t.rearrange("p a b -> p (a b)"),
                    in_=s_d[:, chunk_off[0] * P:(chunk_off[0] + nch) * P],
                )
                idx_off[0] += ssz // 16
                chunk_off[0] += nch

                acc = acc_p.tile([P, 2 * C2], F32, tag="acc")
                npair = nch // 2
                for ci in range(npair):
                    nc.tensor.matmul(
                        acc, lhsT=s_t[:, 2 * ci:2 * ci + 2, :],
                        rhs=g_t[:, 2 * ci:2 * ci + 2, :],
                        start=(ci == 0), stop=(ci == npair - 1 and nch % 2 == 0),
                        perf_mode=mybir.MatmulPerfMode.DoubleRow,
                    )
                if nch % 2 == 1:
                    nc.tensor.matmul(
                        acc, lhsT=s_t[:, nch - 1, :], rhs=g_t[:, nch - 1, :],
                        start=(nch == 1), stop=True,
                    )
                if t == 0:
                    nc.vector.tensor_copy(acc_sb[:, s, :], acc)
                elif t < NSUB - 1:
                    nc.vector.tensor_tensor(
                        out=acc_sb[:, s, :], in0=acc_sb[:, s, :], in1=acc,
                        op=mybir.AluOpType.add,
                    )
                else:
                    sum_sb = p2w.tile([P, 2 * C2], F32, tag="sum")
                    nc.vector.tensor_tensor(
                        out=sum_sb, in0=acc_sb[:, s, :], in1=acc,
                        op=mybir.AluOpType.add,
                    )
                    rec = p2w.tile([P, C2], F32, tag="rec")
                    nc.vector.reciprocal_approx_fast(rec, sum_sb[:, 0:C2])
                    t_sb = p2w.tile([P, C2], BF16, tag="t")
                    nc.vector.tensor_tensor(
                        out=t_sb, in0=sum_sb[:, C2:2 * C2], in1=rec,
                        op=mybir.AluOpType.mult,
                    )
                    tT_sb = p2w.tile([P, 2, P], BF16, tag="tT")
                    nc.sync.dma_start(out=tT_sb, in_=t_sb, transpose=True)
                    o_ps = ops_p.tile([P, C_OUT], F32, tag="o")
                    for k in range(2):
                        nc.tensor.matmul(
                            o_ps, lhsT=tT_sb[:, k, :], rhs=wms[:, k, :],
                            start=(k == 0), stop=(k == 1),
                        )
                    oo = opool.tile([P, C_OUT], F32)
                    nc.vector.tensor_tensor(
                        out=oo, in0=o_ps, in1=bmb, op=mybir.AluOpType.add,
                    )
                    nc.scalar.dma_start(
                        out=out_d[s * SUP:(s + 1) * SUP, :], in_=oo
                    )

            # ---- interleaved schedule ------------------------------------
            for t in range(NSUB):
                for i in range(n_hs[t]):
                    build_hs(TBOUND[t] + i * HS)
                    if t > 0 and i < nsup:
                        process_sup(t - 1, i)
                # spillover if nsup > n_hs[t+...]
                if t > 0:
                    for s in range(n_hs[t], nsup):
                        process_sup(t - 1, s)
            for s in range(nsup):
                process_sup(NSUB - 1, s)

    nc.compile()
    return nc


def _get_prog(nsup, sizes):
    key = (nsup, tuple(map(tuple, sizes)), KQUEUES, KSCRATCH, GBUFS)
    if key not in _PROG_CACHE:
        _PROG_CACHE[key] = build_program(nsup, sizes)
    return _PROG_CACHE[key]


def _wrap16(flat):
    """int16 index layout for dma_gather: [16-partition wrap, replicated x8]."""
    n = flat.shape[0]
    w = flat.reshape(n // 16, 16).T              # [16, n/16]
    return np.tile(w, (8, 1))                    # [128, n/16]


def prep_inputs(features, neighbor_idx, W1, b1, gamma, beta, run_mean, run_var,
                Ws, Wm, bm, n_cores=N_CORES):
    bf16 = ml_dtypes.bfloat16
    f8 = ml_dtypes.float8_e4m3fn

    a = (gamma / np.sqrt(run_var + BN_EPS)).astype(np.float32)
    c = (beta - run_mean * a).astype(np.float32)
    wsp = (a[:, None] * Ws).astype(bf16)
    wmp = (a[:, None] * Wm).astype(bf16)
    bmv = (c.astype(np.float64) @ np.asarray(Wm, np.float64) + bm).astype(np.float32)

    fcm = np.zeros((P, NT), dtype=bf16)
    fcm[:, :N_NODES] = np.asarray(features, np.float32).T.astype(bf16)

    n_pc = neighbor_idx.shape[0] // n_cores
    nodes_pc = -(-n_pc // SUP) * SUP
    nsup = nodes_pc // SUP

    ni_all = np.asarray(neighbor_idx, np.int64)
    # core_edges[core][sup][sub] = (j_rel sorted, n_local)
    core_edges = []
    cnt = np.zeros((NSUB, n_cores, nsup), np.int64)
    for ci in range(n_cores):
        ni = ni_all[ci * n_pc:(ci + 1) * n_pc]
        if nodes_pc != n_pc:
            ni = np.concatenate(
                [ni, np.zeros((nodes_pc - n_pc, K_NBR), np.int64)], axis=0)
        per_sup = []
        for s in range(nsup):
            e = ni[s * SUP:(s + 1) * SUP]
            nloc = np.repeat(np.arange(SUP), K_NBR)
            j = e.reshape(-1)
            subs = []
            for t in range(NSUB):
                m = (j >= TBOUND[t]) & (j < TBOUND[t + 1])
                jt, nt_ = j[m] - TBOUND[t], nloc[m]
                o = np.argsort(jt, kind="stable")   # row-sorted for HBM locality
                subs.append((jt[o], nt_[o]))
                cnt[t, ci, s] = m.sum()
            per_sup.append(subs)
        core_edges.append(per_sup)

    sizes = [
        [int(-(-cnt[t, :, s].max() // P) * P) for s in range(nsup)]
        for t in range(NSUB)
    ]
    for t in range(NSUB):
        for s in range(nsup):
            assert 0 < sizes[t][s] <= GCAP, (t, s, sizes[t][s])

    tot_chunks = sum(sizes[t][s] for t in range(NSUB) for s in range(nsup)) // P
    idx_tot = tot_chunks * P

    shared = dict(
        fcm=fcm, w1=np.ascontiguousarray(W1.astype(bf16)),
        wsp=np.ascontiguousarray(wsp), wmp=np.ascontiguousarray(wmp),
        b1c=np.ascontiguousarray(b1.astype(np.float32).reshape(2, P).T),
        b1r=np.ascontiguousarray(b1.astype(bf16).reshape(1, C2)),
        bmr=bmv.reshape(1, C_OUT),
    )

    in_maps = []
    for ci in range(n_cores):
        idx_cols = []
        s_mat = np.zeros((tot_chunks * P, P), np.uint8)
        chunk_base = 0
        for t in range(NSUB):
            for s in range(nsup):
                jl, nl = core_edges[ci][s][t]
                size = sizes[t][s]
                idx = np.zeros(size, np.int16)
                idx[:len(jl)] = jl.astype(np.int16)
                idx_cols.append(idx)
                base = chunk_base * P
                s_mat[base + np.arange(len(nl)), nl] = 1
                chunk_base += size // P
        idx_flat = np.concatenate(idx_cols)
        assert idx_flat.shape[0] == idx_tot
        idxs = np.ascontiguousarray(_wrap16(idx_flat))
        s_re = s_mat.reshape(tot_chunks, P, P).transpose(1, 0, 2).reshape(
            P, tot_chunks * P)
        smat = s_re.astype(np.float32).astype(f8)
        in_maps.append(dict(shared, idxs=idxs, smat=np.ascontiguousarray(smat)))

    return in_maps, nsup, sizes, n_pc


def kernel(**inputs):
    in_maps, nsup, sizes, n_pc = prep_inputs(**inputs)
    nc = _get_prog(nsup, sizes)
    res = run_bass_kernel_spmd(nc, in_maps, core_ids=list(range(N_CORES)))
    return np.concatenate([r["out"][:n_pc] for r in res.results], axis=0)
